# revision 4
# baseline (speedup 1.0000x reference)
"""Self-contained Trainium2 Bass kernel for nn_Classifier_26723286516422.

Binarized 4-layer CNN on 32x1x224x224 inputs, data-parallel over batch on
8 NeuronCores (4 images each). Strategy per core:

- conv1 (Cout=128, Cin=1, 3x3): im2col in DRAM->SBUF DMA (9 taps on
  partitions, replicated into 4 row-groups at partition bases 0/32/64/96),
  one K=9 matmul per output-row-pair (N=444) with tile_position row packing.
- conv2/3/4: Cin lives on partitions, one accumulating matmul per 3x3 (3x2)
  tap into PSUM.
- LeakyReLU(0.5) is monotonic and per-channel bias is uniform, so both
  commute with maxpool: conv -> maxpool -> Prelu(x + b, alpha=0.5) on ACT.
- maxpool 2x2 from PSUM: either one DVE tensor_reduce(max, axis=XY) over a
  [.., q, row, r] view, or (for a fraction of blocks, to balance DVE vs ACT)
  an ACT Prelu-copy to SBUF fp16 followed by two 2x-mode DVE tensor_tensor
  max ops.
- fp16 activations/weights (weights are exactly +-1), fp32 PSUM accumulate,
  fp32 output.

This file also carries a workaround for the container's walrus build, which
only accepts ONE sync-wait per instruction: excess waits emitted by the Tile
scheduler are split onto single-wait NoOp/Drain carrier instructions.
"""
import sys

sys.path.insert(0, "/opt/trn_rl_repo")

import numpy as np

import concourse.bass as bass
import concourse.mybir as mybir
from concourse.tile import TileContext
from concourse.vector_clock import ScopedClock

F16 = mybir.dt.float16
F32 = mybir.dt.float32
AF = mybir.ActivationFunctionType
MAX = mybir.AluOpType.max

N_IMG = 4
ACTB1 = 0.4
ACTB2 = 0.4

# ---------------------------------------------------------------------------
# walrus single-sync-wait workaround
# ---------------------------------------------------------------------------
_orig_commit_and_lower = TileContext._commit_and_lower
_orig_drain_and_barrier = TileContext._drain_and_barrier


def _split_waits_commit(self, inst, original_block, old_bb_map, bb_to_exit_bb):
    si = inst.sync_info
    if si is not None and len(si.on_wait) > 1:
        waits = list(si.on_wait)
        for w in waits[:-1]:
            nop = mybir.InstNoOp(
                name=self.nc.get_next_instruction_name(),
                engine=inst.engine,
                ins=[],
                outs=[],
                sync_info=mybir.SyncInfo(on_wait=[w], on_update=[]),
            )
            _orig_commit_and_lower(self, nop, original_block, old_bb_map,
                                   bb_to_exit_bb)
        si.on_wait = [waits[-1]]
        inst.sync_info = si
    return _orig_commit_and_lower(self, inst, original_block, old_bb_map,
                                  bb_to_exit_bb)


def _split_drain_and_barrier(self, tick_clock, wait_clock):
    drain_inst = self.nc.sync.drain()
    wait_clock.add_sem_waits(
        drain_inst.ins, ScopedClock({None: tick_clock.global_clock}))
    si = drain_inst.ins.sync_info
    if si is not None and len(si.on_wait) > 1:
        waits = list(si.on_wait)
        si.on_wait = [waits[0]]
        drain_inst.ins.sync_info = si
        for w in waits[1:]:
            d2 = self.nc.sync.drain()
            s2 = d2.ins.sync_info
            if s2 is None:
                s2 = mybir.SyncInfo(on_wait=[], on_update=[])
            s2.on_wait = [w]
            d2.ins.sync_info = s2
    self.nc.all_engine_barrier()
    assert self.sems is not None
    popped = self.nc._tile_sem_poison_stack.pop()
    assert popped is self._sem_poison
    self.nc.clear_and_free_semaphores(list(self.sems.allocated().values()))
    self.nc.all_engine_barrier()


TileContext._commit_and_lower = _split_waits_commit
TileContext._drain_and_barrier = _split_drain_and_barrier


# ---------------------------------------------------------------------------
# kernel builder (per-core program, SPMD across 8 cores)
# ---------------------------------------------------------------------------
def _build(repeat=1):
    nc = bass.Bass(trn_type="TRN2")

    xh = nc.dram_tensor("xh", [N_IMG * 50176 + 64], F16, kind="ExternalInput")
    w1 = nc.dram_tensor("w1sb", [128, 128], F16, kind="ExternalInput")
    w2 = nc.dram_tensor("w2sb", [128, 576], F16, kind="ExternalInput")
    w3 = nc.dram_tensor("w3sb", [64, 288], F16, kind="ExternalInput")
    w4 = nc.dram_tensor("w4sb", [32, 48], F16, kind="ExternalInput")
    b1 = nc.dram_tensor("b1", [128, 1], F32, kind="ExternalInput")
    b2 = nc.dram_tensor("b2", [64, 1], F32, kind="ExternalInput")
    b3 = nc.dram_tensor("b3", [32, 1], F32, kind="ExternalInput")
    b4 = nc.dram_tensor("b4", [8, 1], F32, kind="ExternalInput")
    out = nc.dram_tensor("out", [N_IMG, 4800], F32, kind="ExternalOutput")

    with TileContext(nc) as tc:
        with tc.tile_pool(name="wpool", bufs=1) as wp, \
             tc.tile_pool(name="imcolp", bufs=2) as imp, \
             tc.tile_pool(name="pool1p", bufs=2) as p1p, \
             tc.tile_pool(name="pool2p", bufs=2) as p2p, \
             tc.tile_pool(name="pool3p", bufs=2) as p3p, \
             tc.tile_pool(name="t1bp", bufs=3) as t1bp, \
             tc.tile_pool(name="t1cp", bufs=3) as t1cp, \
             tc.tile_pool(name="t2bp", bufs=3) as t2bp, \
             tc.tile_pool(name="t2cp", bufs=3) as t2cp, \
             tc.tile_pool(name="outp", bufs=2) as outp, \
             tc.tile_pool(name="ps1p", bufs=2, space="PSUM") as ps1p, \
             tc.tile_pool(name="ps2p", bufs=2, space="PSUM") as ps2p, \
             tc.tile_pool(name="ps34p", bufs=2, space="PSUM") as ps34p:

            w1t = wp.tile([128, 128], F16)
            nc.sync.dma_start(w1t[:], w1[:])
            w2t = wp.tile([128, 576], F16)
            nc.sync.dma_start(w2t[:], w2[:])
            w3t = wp.tile([64, 288], F16)
            nc.sync.dma_start(w3t[:], w3[:])
            w4t = wp.tile([32, 48], F16)
            nc.sync.dma_start(w4t[:], w4[:])
            b1t = wp.tile([128, 1], F32)
            nc.sync.dma_start(b1t[:], b1[:])
            b2t = wp.tile([64, 1], F32)
            nc.sync.dma_start(b2t[:], b2[:])
            b3t = wp.tile([32, 1], F32)
            nc.sync.dma_start(b3t[:], b3[:])
            b4t = wp.tile([8, 1], F32)
            nc.sync.dma_start(b4t[:], b4[:])

            def load_imcol(img):
                # conv1 im2col: partition 32g + 3dy + dx holds, for y-pairs
                # p with p % 4 == g at slot p // 4, the 448-elem run
                # x[2p+dy, dx : dx+448] (row pair, 224-wide slots so source
                # runs stay contiguous for the DMA AP balancer).
                imcol = imp.tile([128, 28, 2, 224], F16, tag="imcol",
                                 name="imcol")
                for g in range(4):
                    pairs_g = 28 if g < 3 else 27
                    for dy in range(3):
                        src = bass.AP(
                            xh, img * 50176 + g * 448 + dy * 224,
                            [[1, 3], [1792, pairs_g], [1, 448]],
                        )
                        p0 = 32 * g + 3 * dy
                        nc.sync.dma_start(imcol[p0:p0 + 3, 0:pairs_g], src)
                return imcol

            imcol_next = load_imcol(0)
            for it in range(repeat * N_IMG):
                img = it % N_IMG
                imcol = imcol_next

                # conv1: one matmul (K=9, M=128, N=444) per output-row pair;
                # 2 pairs per 2-bank PSUM tile; row-group packed.
                pooled1 = p1p.tile([128, 111, 112], F16, tag="pooled1")
                for blk in range(56):
                    npair = 2 if blk < 55 else 1
                    ps = ps1p.tile([128, 2, 2, 256], F32, tag="ps1")
                    for j in range(npair):
                        p = 2 * blk + j
                        g = p % 4
                        rhs = imcol[32 * g:32 * g + 9, p // 4, :, 0:222]
                        nc.tensor.matmul(
                            ps[:, j, :, 0:222], w1t[32 * g:32 * g + 9, :], rhs,
                            start=True, stop=True, tile_position=(32 * g, 0),
                        )
                    if ((blk * 7) % 20) / 20.0 < ACTB1:
                        # ACT path: bias+leaky during a de-interleaving
                        # PSUM->SBUF copy, then 2x-mode fp16 DVE max pools.
                        t1b = t1bp.tile([128, 2, 2, 2, 112], F16, tag="t1b")
                        t1c = t1cp.tile([128, 2, 2, 112], F16, tag="t1c")
                        v = ps.rearrange("p a b (q r) -> p a b r q", r=2)
                        nc.scalar.activation(
                            t1b[:, 0:npair, :, :, 0:111],
                            v[:, 0:npair, :, :, 0:111],
                            AF.Prelu, bias=b1t[:], scale=1.0, alpha=0.5)
                        nc.vector.tensor_tensor(
                            t1c[:, 0:npair, :, 0:111],
                            t1b[:, 0:npair, :, 0, 0:111],
                            t1b[:, 0:npair, :, 1, 0:111], MAX)
                        nc.vector.tensor_tensor(
                            pooled1[:, 2 * blk:2 * blk + npair, 0:111],
                            t1c[:, 0:npair, 0, 0:111],
                            t1c[:, 0:npair, 1, 0:111], MAX)
                    else:
                        # DVE path: whole 2x2 pool in one tensor_reduce.
                        w = ps.rearrange("p a b (q r) -> p a q b r", r=2)
                        nc.vector.tensor_reduce(
                            pooled1[:, 2 * blk:2 * blk + npair, 0:111],
                            w[:, 0:npair, 0:111, :, :],
                            mybir.AxisListType.XY, MAX)
                        nc.scalar.activation(
                            pooled1[:, 2 * blk:2 * blk + npair, 0:111],
                            pooled1[:, 2 * blk:2 * blk + npair, 0:111],
                            AF.Prelu, bias=b1t[:], scale=1.0, alpha=0.5)

                if it + 1 < repeat * N_IMG:
                    imcol_next = load_imcol((img + 1) % N_IMG)

                # conv2: 27 blocks x 4 conv rows; 9 accumulating matmuls.
                pooled2 = p2p.tile([64, 54, 56], F16, tag="pooled2")
                for blk in range(27):
                    y0 = 4 * blk
                    Y0 = y0 // 2
                    ps = ps2p.tile([64, 4, 128], F32, tag="ps2")
                    for t in range(9):
                        dy, dx = divmod(t, 3)
                        rhs = pooled1[:, y0 + dy:y0 + dy + 4, dx:dx + 108]
                        nc.tensor.matmul(
                            ps[:, :, 0:108], w2t[:, 64 * t:64 * t + 64], rhs,
                            start=(t == 0), stop=(t == 8))
                    if ((blk * 7) % 20) / 20.0 < ACTB2:
                        t2b = t2bp.tile([64, 4, 2, 56], F16, tag="t2b")
                        t2c = t2cp.tile([64, 4, 56], F16, tag="t2c")
                        v = ps.rearrange("p y (q r) -> p y r q", r=2)
                        nc.scalar.activation(
                            t2b[:, :, :, 0:54], v[:, :, :, 0:54],
                            AF.Prelu, bias=b2t[:], scale=1.0, alpha=0.5)
                        nc.vector.tensor_tensor(
                            t2c[:, :, 0:54],
                            t2b[:, :, 0, 0:54], t2b[:, :, 1, 0:54], MAX)
                        r = t2c.rearrange("p (u v) q -> p u v q", v=2)
                        nc.vector.tensor_tensor(
                            pooled2[:, Y0:Y0 + 2, 0:54],
                            r[:, :, 0, 0:54], r[:, :, 1, 0:54], MAX)
                    else:
                        w = ps.rearrange("p (u v) (q r) -> p u q v r", v=2, r=2)
                        nc.vector.tensor_reduce(
                            pooled2[:, Y0:Y0 + 2, 0:54],
                            w[:, :, 0:54, :, :], mybir.AxisListType.XY, MAX)
                        nc.scalar.activation(
                            pooled2[:, Y0:Y0 + 2, 0:54],
                            pooled2[:, Y0:Y0 + 2, 0:54],
                            AF.Prelu, bias=b2t[:], scale=1.0, alpha=0.5)

                # conv3: blocks of ny=8 conv rows (last ny=4).
                pooled3 = p3p.tile([32, 26, 28], F16, tag="pooled3")
                for blk in range(7):
                    y0 = 8 * blk
                    ny = 8 if blk < 6 else 4
                    ps = ps34p.tile([32, 8, 64], F32, tag="ps34", name="ps3")
                    for t in range(9):
                        dy, dx = divmod(t, 3)
                        rhs = pooled2[:, y0 + dy:y0 + dy + ny, dx:dx + 52]
                        nc.tensor.matmul(
                            ps[:, 0:ny, 0:52], w3t[:, 32 * t:32 * t + 32], rhs,
                            start=(t == 0), stop=(t == 8))
                    w = ps.rearrange("p (u v) (q r) -> p u q v r", v=2, r=2)
                    Y0 = y0 // 2
                    nc.vector.tensor_reduce(
                        pooled3[:, Y0:Y0 + ny // 2, 0:26],
                        w[:, 0:ny // 2, 0:26, :, :], mybir.AxisListType.XY, MAX)
                    nc.scalar.activation(
                        pooled3[:, Y0:Y0 + ny // 2, 0:26],
                        pooled3[:, Y0:Y0 + ny // 2, 0:26],
                        AF.Prelu, bias=b3t[:], scale=1.0, alpha=0.5)

                # conv4: 2 blocks x 12 rows, taps 3x2, bias only.
                out_sb = outp.tile([8, 600], F32, tag="out_sb")
                o = out_sb.rearrange("p (y x) -> p y x", x=25)
                for blk in range(2):
                    y0 = 12 * blk
                    ps = ps34p.tile([8, 12, 32], F32, tag="ps34", name="ps4")
                    for t in range(6):
                        dy, dx = divmod(t, 2)
                        rhs = pooled3[:, y0 + dy:y0 + dy + 12, dx:dx + 25]
                        nc.tensor.matmul(
                            ps[:, :, 0:25], w4t[:, 8 * t:8 * t + 8], rhs,
                            start=(t == 0), stop=(t == 5))
                    nc.scalar.activation(
                        o[:, y0:y0 + 12, :], ps[:, :, 0:25],
                        AF.Identity, bias=b4t[:], scale=1.0)

                dst = out[img].rearrange("(p f) -> p f", p=8)
                nc.sync.dma_start(dst, out_sb[:])

    return nc


# ---------------------------------------------------------------------------
# host-side entry point
# ---------------------------------------------------------------------------
_NC_CACHE = {}
LAST_EXEC_NS = None


def _get_nc():
    if "nc" not in _NC_CACHE:
        _NC_CACHE["nc"] = _build()
    return _NC_CACHE["nc"]


def _binarize(w):
    return np.where(np.asarray(w, np.float32) >= 0, 1.0, -1.0).astype(np.float16)


def _prep_weights(w1, b1, w2, b2, w3, b3, w4, b4):
    w1T = _binarize(w1).reshape(128, 9).T  # [9, 128]
    w1sb = np.zeros((128, 128), np.float16)
    for g in range(4):
        w1sb[32 * g:32 * g + 9, :] = w1T

    wb2 = _binarize(w2)  # [64, 128, 3, 3]
    w2sb = np.zeros((128, 576), np.float16)
    for t in range(9):
        dy, dx = divmod(t, 3)
        w2sb[:, 64 * t:64 * t + 64] = wb2[:, :, dy, dx].T

    wb3 = _binarize(w3)  # [32, 64, 3, 3]
    w3sb = np.zeros((64, 288), np.float16)
    for t in range(9):
        dy, dx = divmod(t, 3)
        w3sb[:, 32 * t:32 * t + 32] = wb3[:, :, dy, dx].T

    wb4 = _binarize(w4)  # [8, 32, 3, 2]
    w4sb = np.zeros((32, 48), np.float16)
    for t in range(6):
        dy, dx = divmod(t, 2)
        w4sb[:, 8 * t:8 * t + 8] = wb4[:, :, dy, dx].T

    return {
        "w1sb": w1sb, "w2sb": w2sb, "w3sb": w3sb, "w4sb": w4sb,
        "b1": np.asarray(b1, np.float32).reshape(128, 1),
        "b2": np.asarray(b2, np.float32).reshape(64, 1),
        "b3": np.asarray(b3, np.float32).reshape(32, 1),
        "b4": np.asarray(b4, np.float32).reshape(8, 1),
    }


def kernel(x, w1, b1, w2, b2, w3, b3, w4, b4):
    global LAST_EXEC_NS
    from concourse.bass_utils import run_bass_kernel_spmd

    nc = _get_nc()
    wmap = _prep_weights(w1, b1, w2, b2, w3, b3, w4, b4)

    x = np.asarray(x, np.float32)  # [32, 1, 224, 224]
    in_maps = []
    for c in range(8):
        shard = x[4 * c:4 * c + 4, 0].astype(np.float16).ravel()
        m = dict(wmap)
        m["xh"] = np.concatenate([shard, np.zeros(64, np.float16)])
        in_maps.append(m)

    res = run_bass_kernel_spmd(nc, in_maps, core_ids=list(range(8)))
    LAST_EXEC_NS = res.exec_time_ns
    out = np.concatenate([res.results[c]["out"] for c in range(8)], axis=0)
    return np.ascontiguousarray(out.astype(np.float32))

